# revision 1
# baseline (speedup 1.0000x reference)
"""Two-layer GCN + linear head on 8 Trainium2 NeuronCores (Bass/Tile).

Math (per GCN layer, PyG GCNConv with self loops, symmetric norm):
    deg[c]  = 1 + |{e : col_e == c}|          (self loop counted)
    dinv    = 1/sqrt(deg)
    u       = dinv * (x @ W)                  (row-wise pre-scale)
    out[c]  = sum_{e->c} dinv[c] * u[row_e] + dinv[c] * u[c] + b
    x1      = relu(out)

Device mapping:
  - Nodes padded to a multiple of 8*128; dst blocks of 128 nodes sharded
    round-robin-contiguous across 8 cores (49 blocks/core for N=50000).
  - Dense phase sharded: each core computes u for its own blocks, writes a
    bf16 hi/lo pair table row [u_hi | u_lo] (512B/row), AllGather -> full
    table on every core.
  - Scatter phase: edges sorted by dst block; per 128-edge tile, dma_gather
    pulls u_pair rows; a one-hot matrix O'[e,d] = dinv[col_e]*(col_local_e==d)
    is built in ONE DVE op (tensor_scalar is_equal+mult); two bf16 matmuls
    (hi,lo stationary) accumulate out^T[f,d] in PSUM across the block's tiles.
    Self loops are appended as ordinary edges. No DRAM read-modify-write.
  - Transposed accumulation [f,d] feeds layer-2 dense and the head directly
    as matmul stationary operands (no transposes anywhere).
  - int16 gather indices => node space split in two tables (A: < SPLIT,
    B: >= SPLIT) with per-(block,stream) uniform tile slots so the program
    is identical across cores (SPMD).

Host does only index-side prep (shard/sort/pad edge lists, integer degree
counts) — all float math runs on device.
"""
import os
import sys

sys.path.insert(0, "/opt/trn_rl_repo")

import numpy as np

import ml_dtypes

P = 128
NCORES = 8


def _ceil_div(a, b):
    return (a + b - 1) // b


def _prep(x, edge_index):
    """Host-side index prep. Returns per-core input dicts + metadata."""
    N, D = x.shape
    assert D == P
    E = edge_index.shape[1]
    NB = _ceil_div(N, P)
    NB = _ceil_div(NB, NCORES) * NCORES  # blocks multiple of 8
    Npad = NB * P
    NBC = NB // NCORES
    # split node space at a block boundary, both halves must fit int16
    SPLIT = (NB // 2) * P
    assert SPLIT <= 32768 and Npad - SPLIT <= 32768, (Npad, SPLIT)

    row = np.asarray(edge_index[0], dtype=np.int64)
    col = np.asarray(edge_index[1], dtype=np.int64)

    deg = np.bincount(col, minlength=Npad).astype(np.int64) + 1  # + self loop
    deg[N:] = 1

    blk = (col >> 7).astype(np.int64)

    # order edges by (block, table, src) once globally
    table = (row >= SPLIT).astype(np.int64)
    order = np.lexsort((row, table, blk))
    row_s, col_s, blk_s, table_s = row[order], col[order], blk[order], table[order]
    blk_start = np.searchsorted(blk_s, np.arange(NB + 1))

    # per (block, stream) edge lists with self-loops appended
    per_blk = []  # (rowsA, colsA, rowsB, colsB) local col + global row
    for b in range(NB):
        lo, hi = blk_start[b], blk_start[b + 1]
        t = table_s[lo:hi]
        mid = lo + int(np.searchsorted(t, 1))
        rA, cA = row_s[lo:mid], col_s[lo:mid] & 127
        rB, cB = row_s[mid:hi], col_s[mid:hi] & 127
        selfr = b * P + np.arange(P, dtype=np.int64)
        selfc = np.arange(P, dtype=np.int64)
        if b * P < SPLIT:
            rA = np.concatenate([rA, selfr])
            cA = np.concatenate([cA, selfc])
        else:
            rB = np.concatenate([rB, selfr])
            cB = np.concatenate([cB, selfc])
        per_blk.append((rA, cA, rB, cB))

    SA = max(_ceil_div(len(t[0]), P) for t in per_blk)
    SB = max(_ceil_div(len(t[2]), P) for t in per_blk)

    def pad_to(a, n, val):
        out = np.full(n, val, dtype=np.int64)
        out[: len(a)] = a
        return out

    # build per-core streams
    cores = []
    for c in range(NCORES):
        idxA = np.zeros(NBC * SA * P, dtype=np.int64)
        colA = np.full(NBC * SA * P, 999, dtype=np.int64)
        idxB = np.zeros(NBC * SB * P, dtype=np.int64)
        colB = np.full(NBC * SB * P, 999, dtype=np.int64)
        for i in range(NBC):
            b = c * NBC + i
            rA, cA, rB, cB = per_blk[b]
            idxA[i * SA * P : i * SA * P + len(rA)] = rA
            colA[i * SA * P : i * SA * P + len(cA)] = cA
            idxB[i * SB * P : i * SB * P + len(rB)] = rB - SPLIT
            colB[i * SB * P : i * SB * P + len(cB)] = cB
        # degree of the col for each lane (pad lanes -> 1)
        dcolA = np.ones(NBC * SA * P, dtype=np.float32)
        dcolB = np.ones(NBC * SB * P, dtype=np.float32)
        mA = colA < P
        mB = colB < P
        baseA = (np.arange(NBC * SA * P) // (SA * P) + c * NBC) * P
        baseB = (np.arange(NBC * SB * P) // (SB * P) + c * NBC) * P
        dcolA[mA] = deg[baseA[mA] + colA[mA]]
        dcolB[mB] = deg[baseB[mB] + colB[mB]]

        def wrap16(v):  # unwrapped[i] -> [i % 16, i // 16], replicated to 128 rows
            w = v.astype(np.int16).reshape(-1, 16).T  # [16, n/16]
            return np.tile(w, (8, 1)).copy()

        def lanes(v, dt):  # [ntiles*128] -> [128, ntiles] (lane-major columns)
            return np.ascontiguousarray(v.reshape(-1, P).T.astype(dt))

        colstream = np.concatenate([colA, colB])
        degstream = np.concatenate([dcolA, dcolB])
        own = slice(c * NBC * P, (c + 1) * NBC * P)
        deg_own = deg[own].astype(np.float32).reshape(NBC, P).T  # [128, NBC]

        xpad = np.zeros((NBC * P, P), dtype=np.float32)
        realn = min(max(N - c * NBC * P, 0), NBC * P)
        xpad[:realn] = x[c * NBC * P : c * NBC * P + realn]

        cores.append(
            dict(
                xT_shard=np.ascontiguousarray(xpad.T),  # [128, NBC*128] f32
                idxA=wrap16(idxA),
                idxB=wrap16(idxB),
                colstream=lanes(colstream, np.float32),  # [128, NBC*(SA+SB)]
                degstream=lanes(degstream, np.float32),
                deg_own=np.ascontiguousarray(deg_own),  # [128, NBC]
            )
        )
    meta = dict(N=N, Npad=Npad, NB=NB, NBC=NBC, SA=SA, SB=SB, SPLIT=SPLIT)
    return cores, meta


def _build_program(meta, with_bias_gcn):
    """Emit the SPMD bass program (identical for all cores)."""
    from concourse import bacc, mybir
    from concourse.tile import TileContext
    from contextlib import ExitStack

    f32 = mybir.dt.float32
    bf16 = mybir.dt.bfloat16
    i16 = mybir.dt.int16
    i32 = mybir.dt.int32
    AF = mybir.ActivationFunctionType
    OP = mybir.AluOpType

    NBC, SA, SB, SPLIT, Npad = (
        meta["NBC"], meta["SA"], meta["SB"], meta["SPLIT"], meta["Npad"]
    )
    S = SA + SB
    nA, nB = NBC * SA, NBC * SB  # total A/B tiles per core per layer

    # gather chunking: <= 8 tiles (1024 idxs) per dma_gather call
    # (>1024 idxs per SWDGE gather crashes the exec unit: NRT status 101)
    def chunks(ntiles):
        per = min(8, ntiles)
        while ntiles % per:
            per -= 1
        return per

    CA, CB = chunks(nA), chunks(nB)

    nc = bacc.Bacc("TRN2", target_bir_lowering=False, num_devices=NCORES, dynamic_dma_scratch_size=131072)

    xT = nc.declare_dram_parameter("xT_shard", [P, NBC * P], f32, isOutput=False)
    W1d = nc.declare_dram_parameter("W1", [P, P], f32, isOutput=False)
    W2d = nc.declare_dram_parameter("W2", [P, P], f32, isOutput=False)
    Wld = nc.declare_dram_parameter("Wl", [2 * P, P], f32, isOutput=False)
    b1d = nc.declare_dram_parameter("b1", [1, P], f32, isOutput=False)
    b2d = nc.declare_dram_parameter("b2", [1, P], f32, isOutput=False)
    bld = nc.declare_dram_parameter("bl", [1, P], f32, isOutput=False)
    idxAd = nc.declare_dram_parameter("idxA", [P, nA * P // 16], i16, isOutput=False)
    idxBd = nc.declare_dram_parameter("idxB", [P, nB * P // 16], i16, isOutput=False)
    cold = nc.declare_dram_parameter("colstream", [P, NBC * S], f32, isOutput=False)
    degd = nc.declare_dram_parameter("degstream", [P, NBC * S], f32, isOutput=False)
    degod = nc.declare_dram_parameter("deg_own", [P, NBC], f32, isOutput=False)
    outd = nc.declare_dram_parameter("out_shard", [NBC * P, P], f32, isOutput=True)

    ag_in = [nc.dram_tensor(f"ag{i}_in", [NBC * P, 2 * P], bf16) for i in (1, 2)]
    ag_out = [
        nc.dram_tensor(f"ag{i}_out", [Npad, 2 * P], bf16, addr_space="Shared")
        for i in (1, 2)
    ]

    def _emit(tc, ctx):
        const = ctx.enter_context(tc.tile_pool(name="const", bufs=1))
        sb = ctx.enter_context(tc.tile_pool(name="sb", bufs=3))
        gbufs = ctx.enter_context(tc.tile_pool(name="gbufs", bufs=2))
        obuf = ctx.enter_context(tc.tile_pool(name="obuf", bufs=6))
        psum = ctx.enter_context(tc.tile_pool(name="psum", bufs=4, space="PSUM"))
        psd = ctx.enter_context(tc.tile_pool(name="psd", bufs=2, space="PSUM"))

        # --- constants / streams ---
        iota_i = const.tile([P, P], i32)
        nc.gpsimd.iota(iota_i[:], pattern=[[1, P]], base=0, channel_multiplier=0)
        iota_f = const.tile([P, P], f32)
        nc.vector.tensor_copy(out=iota_f[:], in_=iota_i[:])

        W1 = const.tile([P, P], f32)
        W2 = const.tile([P, P], f32)
        Wl = const.tile([P, 2 * P], f32)
        nc.sync.dma_start(out=W1[:], in_=W1d[:])
        nc.sync.dma_start(out=W2[:], in_=W2d[:])
        nc.sync.dma_start(out=Wl[:, 0:P], in_=Wld[0:P, :])
        nc.sync.dma_start(out=Wl[:, P : 2 * P], in_=Wld[P : 2 * P, :])

        # bias tiles (row 0 = bias vector), ones row tile
        onesrow = const.tile([P, P], f32)
        nc.vector.memset(onesrow[:], 0.0)
        nc.vector.memset(onesrow[0:1, :], 1.0)
        btile = []
        for bd in (b1d, b2d, bld):
            t = const.tile([P, P], f32, tag="bias")
            nc.vector.memset(t[:], 0.0)
            nc.sync.dma_start(out=t[0:1, :], in_=bd[:])
            btile.append(t)

        idxA = const.tile([P, nA * P // 16], i16)
        idxB = const.tile([P, nB * P // 16], i16)
        nc.sync.dma_start(out=idxA[:], in_=idxAd[:])
        nc.sync.dma_start(out=idxB[:], in_=idxBd[:])

        colst = const.tile([P, NBC * S], f32)
        nc.sync.dma_start(out=colst[:], in_=cold[:])
        dinvc = const.tile([P, NBC * S], f32)
        nc.sync.dma_start(out=dinvc[:], in_=degd[:])
        nc.scalar.activation(out=dinvc[:], in_=dinvc[:], func=AF.Sqrt)
        nc.vector.reciprocal(out=dinvc[:], in_=dinvc[:])

        dinvo = const.tile([P, NBC], f32)
        nc.sync.dma_start(out=dinvo[:], in_=degod[:])
        nc.scalar.activation(out=dinvo[:], in_=dinvo[:], func=AF.Sqrt)
        nc.vector.reciprocal(out=dinvo[:], in_=dinvo[:])

        # persistent x1^T blocks [f, d] for layer-2 dense + head
        x1T = const.tile([P, NBC * P], f32)

        def dense_block(b, src_lhsT, W, layer):
            """u_pair[b] = split(dinv_own[b] * (x_b @ W)) -> ag_in[layer]"""
            ps = psd.tile([P, P], f32, space="PSUM", tag="psd")
            nc.tensor.matmul(ps[:], lhsT=src_lhsT, rhs=W[:], start=True, stop=True)
            t = sb.tile([P, P], f32, tag="dense_t")
            nc.vector.tensor_scalar(
                out=t[:], in0=ps[:], scalar1=dinvo[:, b : b + 1],
                scalar2=None, op0=OP.mult,
            )
            pair = sb.tile([P, 2 * P], bf16, tag="dense_pair")
            nc.vector.tensor_copy(out=pair[:, 0:P], in_=t[:])
            nc.vector.tensor_tensor(
                out=pair[:, P : 2 * P], in0=t[:], in1=pair[:, 0:P], op=OP.subtract
            )
            nc.sync.dma_start(
                out=ag_in[layer][b * P : (b + 1) * P, :], in_=pair[:]
            )

        def scatter_layer(layer, post_fn):
            """Message passing for one layer; post_fn(b, psum_tile) consumes
            the accumulated transposed block. Gather calls are emitted lazily,
            right before the first tile that consumes them, so A/B chunk
            issue order interleaves with consumption (no buffer-wait cycles)."""
            issued = [{}, {}]  # stream -> chunk_id -> gbuf tile

            def tile_src(stream, t):
                CH = CA if stream == 0 else CB
                cid = t // CH
                if cid not in issued[stream]:
                    idxs = idxA if stream == 0 else idxB
                    if stream == 0:
                        tbl = ag_out[layer][0:SPLIT, :]
                    else:
                        tbl = ag_out[layer][SPLIT:Npad, :]
                    g = gbufs.tile([P, CH, 2 * P], bf16, tag=f"g{stream}")
                    c0 = cid * CH
                    nc.gpsimd.dma_gather(
                        out_ap=g[:],
                        in_ap=tbl,
                        idxs_ap=idxs[:, c0 * 8 : (c0 + CH) * 8],
                        num_idxs=CH * P,
                        num_idxs_reg=CH * P,
                        elem_size=2 * P,
                    )
                    issued[stream][cid] = g
                g = issued[stream][cid]
                return g[:, t % CH, :]

            for b in range(NBC):
                acc = psum.tile([P, P], f32, space="PSUM", tag="acc")
                first = True
                for stream, s_cnt in ((0, SA), (1, SB)):
                    for s in range(s_cnt):
                        t = b * s_cnt + s
                        gt = (b * SA + s) if stream == 0 else (nA + b * SB + s)
                        gsl = tile_src(stream, t)
                        o = obuf.tile([P, P], bf16, tag="onehot")
                        nc.vector.tensor_scalar(
                            out=o[:],
                            in0=iota_f[:],
                            scalar1=colst[:, gt : gt + 1],
                            scalar2=dinvc[:, gt : gt + 1],
                            op0=OP.is_equal,
                            op1=OP.mult,
                        )
                        last = (stream == 1) and (s == s_cnt - 1) and not with_bias_gcn
                        nc.tensor.matmul(
                            acc[:], lhsT=gsl[0:P, 0:P], rhs=o[:],
                            start=first, stop=False,
                        )
                        nc.tensor.matmul(
                            acc[:], lhsT=gsl[0:P, P : 2 * P], rhs=o[:],
                            start=False, stop=last,
                        )
                        first = False
                if with_bias_gcn:
                    nc.tensor.matmul(
                        acc[:], lhsT=btile[layer][:], rhs=onesrow[:],
                        start=False, stop=True,
                    )
                post_fn(b, acc)

        phase = os.environ.get("KERNEL_PHASE", "full")

        # ---------- layer 1 dense ----------
        for b in range(NBC):
            lx = sb.tile([P, P], f32, tag="xT_in")
            nc.sync.dma_start(out=lx[:], in_=xT[:, b * P : (b + 1) * P])
            dense_block(b, lx[:], W1, 0)
        if phase == "dense":
            for b in range(NBC):
                z = sb.tile([P, P], f32, tag="out_t")
                nc.vector.memset(z[:], 0.0)
                nc.sync.dma_start(out=outd[b * P : (b + 1) * P, :], in_=z[:])
            return
        nc.gpsimd.collective_compute(
            "AllGather", mybir.AluOpType.bypass,
            replica_groups=[list(range(NCORES))],
            ins=[ag_in[0][:]], outs=[ag_out[0][:]],
        )
        if phase == "ag1":
            for b in range(NBC):
                z = sb.tile([P, P], f32, tag="out_t")
                nc.sync.dma_start(out=z[:], in_=ag_out[0].bitcast(f32)[b * P : (b + 1) * P, :])
                nc.sync.dma_start(out=outd[b * P : (b + 1) * P, :], in_=z[:])
            return

        # ---------- layer 1 scatter -> x1T ----------
        def post1(b, acc):
            nc.scalar.activation(
                out=x1T[:, b * P : (b + 1) * P], in_=acc[:], func=AF.Relu
            )
            if phase != "l1nd":
                dense_block(b, x1T[:, b * P : (b + 1) * P], W2, 1)

        scatter_layer(0, post1)
        if phase in ("l1", "l1nd"):
            for b in range(NBC):
                z = sb.tile([P, P], f32, tag="out_t")
                nc.vector.tensor_copy(out=z[:], in_=x1T[:, b * P : (b + 1) * P])
                nc.sync.dma_start(out=outd[b * P : (b + 1) * P, :], in_=z[:])
            return
        nc.gpsimd.collective_compute(
            "AllGather", mybir.AluOpType.bypass,
            replica_groups=[list(range(NCORES))],
            ins=[ag_in[1][:]], outs=[ag_out[1][:]],
        )

        # ---------- layer 2 scatter -> head ----------
        def post2(b, acc):
            x2T = sb.tile([P, P], f32, tag="x2T")
            nc.scalar.activation(out=x2T[:], in_=acc[:], func=AF.Relu)
            ph = psd.tile([P, P], f32, space="PSUM", tag="ph")
            nc.tensor.matmul(
                ph[:], lhsT=x1T[:, b * P : (b + 1) * P], rhs=Wl[:, 0:P],
                start=True, stop=False,
            )
            nc.tensor.matmul(
                ph[:], lhsT=x2T[:], rhs=Wl[:, P : 2 * P], start=False, stop=False
            )
            nc.tensor.matmul(
                ph[:], lhsT=onesrow[:], rhs=btile[2][:], start=False, stop=True
            )
            ot = sb.tile([P, P], f32, tag="out_t")
            nc.vector.tensor_copy(out=ot[:], in_=ph[:])
            nc.sync.dma_start(out=outd[b * P : (b + 1) * P, :], in_=ot[:])

        scatter_layer(1, post2)

    with TileContext(nc) as tc, ExitStack() as ctx:
        _emit(tc, ctx)

    nc.compile()
    return nc


def kernel(x, edge_index, W1, b1, W2, b2, Wl, bl):
    x = np.asarray(x, dtype=np.float32)
    cores, meta = _prep(x, np.asarray(edge_index))
    with_bias_gcn = bool(np.any(b1) or np.any(b2))

    nc = _build_program(meta, with_bias_gcn)

    shared = dict(
        W1=np.asarray(W1, np.float32),
        W2=np.asarray(W2, np.float32),
        Wl=np.asarray(Wl, np.float32),
        b1=np.asarray(b1, np.float32).reshape(1, P),
        b2=np.asarray(b2, np.float32).reshape(1, P),
        bl=np.asarray(bl, np.float32).reshape(1, P),
    )
    in_maps = [{**c, **shared} for c in cores]
    N = meta["N"]

    if os.environ.get("KERNEL_SIM"):
        from concourse.bass_interp import MultiCoreSim

        sim = MultiCoreSim(nc, NCORES)
        for i in range(NCORES):
            for k, v in in_maps[i].items():
                sim.cores[i].tensor(k)[:] = v
        sim.simulate()
        out = np.concatenate(
            [np.asarray(sim.cores[i].tensor("out_shard")) for i in range(NCORES)],
            axis=0,
        )
        return np.ascontiguousarray(out[:N])

    from concourse.bass_utils import run_bass_kernel_spmd

    trace = bool(int(os.environ.get("KERNEL_TRACE", "0")))
    if trace:
        try:
            import ntff_shim  # noqa: F401
        except ImportError:
            pass

    br = run_bass_kernel_spmd(nc, in_maps, list(range(NCORES)), trace=trace)
    kernel.last_result = br

    out = np.concatenate([r["out_shard"] for r in br.results], axis=0)
    return np.ascontiguousarray(out[:N])

